# revision 11
# baseline (speedup 1.0000x reference)
"""DirectionalContrastiveLoss on 8 TRN2 NeuronCores (Bass/Tile).

Data-parallel over the N=16384 anchor rows (2048 rows/core); the 4000-row
memory bank is replicated. The label-inequality mask is fused into the
matmul: anchors are extended with -1000*onehot(label) rows and the memory
bank with onehot(mem_label) rows, so the PE directly produces
sim - 1000*eq.  exp(sim - 1000) == 0 in fp32, which reproduces the masked
exp-sum exactly.  Row-softmax uses a per-unit coarse max (first 500 of each
1000-column unit, pos folded in) with online rescaling; overflow to +inf is
benign (the row is provably dead: its -log reduces to -log(EPS)).
Each core returns [128, 4] partial sums (num1, den1, num2, den2); the host
does the final cross-core reduction and division.
"""
from contextlib import ExitStack

import numpy as np
import ml_dtypes

TEMP = 0.1
POS_THRESH = 0.7
EPS = 1e-8
N, C, M, NLAB = 16384, 256, 4000, 21
KEXT = C + NLAB            # 277 contraction rows
NCORES = 8
RPC = N // NCORES          # 2048 rows per core
NT = RPC // 128            # 16 n-tiles per core
NU = 4                     # psum units per n-tile (1000 cols each)
UNIT = M // NU             # 1000
SUB = 500                  # coarse-max subset width per unit

_cache = {}


def _build():
    import concourse.bass as bass
    import concourse.bacc as bacc
    import concourse.tile as tile
    from concourse import mybir

    f32 = mybir.dt.float32
    bf16 = mybir.dt.bfloat16
    Alu = mybir.AluOpType
    Act = mybir.ActivationFunctionType
    X = mybir.AxisListType.X

    # Bacc (not raw Bass): its finalize() runs generate_event_semaphores(),
    # which splits multi-sem waits into EVSEM chains — walrus allows at most
    # one sync-wait per instruction.
    nc = bacc.Bacc(None)

    ext1_d = nc.declare_dram_parameter("ext1", [KEXT, RPC], bf16, isOutput=False)
    ext2_d = nc.declare_dram_parameter("ext2", [KEXT, RPC], bf16, isOutput=False)
    mem_d = nc.declare_dram_parameter("extmem", [KEXT, M], bf16, isOutput=False)
    f1_d = nc.declare_dram_parameter("f1r", [128, NT * C], bf16, isOutput=False)
    f2_d = nc.declare_dram_parameter("f2r", [128, NT * C], bf16, isOutput=False)
    lg1_d = nc.declare_dram_parameter("lg1", [128, NT], f32, isOutput=False)
    lg2_d = nc.declare_dram_parameter("lg2", [128, NT], f32, isOutput=False)
    out_d = nc.declare_dram_parameter("out", [128, 4], f32, isOutput=True)

    ksplits = [(0, 128), (128, 256), (256, KEXT)]

    with tile.TileContext(nc) as tc, ExitStack() as ctx:
        consts = ctx.enter_context(tc.tile_pool(name="consts", bufs=1))
        small = ctx.enter_context(tc.tile_pool(name="small", bufs=3))
        psum = ctx.enter_context(
            tc.tile_pool(name="psum", bufs=NU, space="PSUM")
        )

        # ---- resident inputs ----
        mem_k, e1_k, e2_k = [], [], []
        for i, (k0, k1) in enumerate(ksplits):
            kp = k1 - k0
            mt = consts.tile([kp, M], bf16, tag=f"mem{i}")
            nc.sync.dma_start(out=mt[:], in_=mem_d[k0:k1, :])
            mem_k.append(mt)
            t1 = consts.tile([kp, RPC], bf16, tag=f"e1_{i}")
            nc.sync.dma_start(out=t1[:], in_=ext1_d[k0:k1, :])
            e1_k.append(t1)
            t2 = consts.tile([kp, RPC], bf16, tag=f"e2_{i}")
            nc.sync.dma_start(out=t2[:], in_=ext2_d[k0:k1, :])
            e2_k.append(t2)

        f1t = consts.tile([128, NT, C], bf16, tag="f1t")
        nc.sync.dma_start(out=f1t[:], in_=f1_d[:].rearrange("p (t c) -> p t c", c=C))
        f2t = consts.tile([128, NT, C], bf16, tag="f2t")
        nc.sync.dma_start(out=f2t[:], in_=f2_d[:].rearrange("p (t c) -> p t c", c=C))
        lg1t = consts.tile([128, NT], f32, tag="lg1t")
        nc.sync.dma_start(out=lg1t[:], in_=lg1_d[:])
        lg2t = consts.tile([128, NT], f32, tag="lg2t")
        nc.sync.dma_start(out=lg2t[:], in_=lg2_d[:])

        outt = consts.tile([128, 4], f32, tag="outt")
        epsb = consts.tile([128, 1], f32, tag="epsb")
        nc.vector.memset(epsb[:], EPS)

        # pos is shared by both branches: pos = sum_c (f1/TEMP)*f2
        # (the 1/TEMP scale is folded into f1r host-side)
        POS = consts.tile([128, NT], f32, tag="POS")
        for t in range(NT):
            scr = small.tile([128, C], f32, tag="posscr")
            nc.vector.tensor_mul(scr[:], f1t[:, t, :], f2t[:, t, :])
            nc.vector.reduce_sum(out=POS[:, t : t + 1], in_=scr[:], axis=X)

        for b, (ekt, lgA, lgB) in enumerate(
            [(e1_k, lg1t, lg2t), (e2_k, lg2t, lg1t)]
        ):
            SS = consts.tile([128, NT], f32, tag=f"SS{b}")
            QQ = consts.tile([128, NT], f32, tag=f"QQ{b}")
            for t in range(NT):
                pu = [
                    psum.tile([128, 2, 512], f32, tag="pu", name=f"pu{b}_{t}_{u}")
                    for u in range(NU)
                ]
                for kt in range(3):
                    lhsT = ekt[kt][:, t * 128 : (t + 1) * 128]
                    for u in range(NU):
                        for j in range(2):
                            nc.tensor.matmul(
                                pu[u][:, j, 0:SUB],
                                lhsT,
                                mem_k[kt][:, u * UNIT + j * SUB : u * UNIT + (j + 1) * SUB],
                                start=(kt == 0),
                                stop=(kt == 2),
                            )
                MB = small.tile([128, NU], f32, tag="MB")
                NB = small.tile([128, NU], f32, tag="NB")
                S = small.tile([128, NU], f32, tag="S")
                for u in range(NU):
                    # coarse max over first SUB cols of the unit, pos folded in
                    nc.vector.reduce_max(
                        out=MB[:, u : u + 1], in_=pu[u][:, 0, 0:SUB], axis=X
                    )
                    nc.vector.tensor_max(
                        MB[:, u : u + 1], MB[:, u : u + 1], POS[:, t : t + 1]
                    )
                    nc.vector.tensor_scalar_mul(
                        NB[:, u : u + 1], MB[:, u : u + 1], -1.0
                    )
                    # exp(sim - B_u) in place; S_u = row-sum
                    nc.scalar.activation(
                        out=pu[u][:, :, 0:SUB],
                        in_=pu[u][:, :, 0:SUB],
                        func=Act.Exp,
                        bias=NB[:, u : u + 1],
                        scale=1.0,
                        accum_out=S[:, u : u + 1],
                    )
                # online combine across units
                mh = small.tile([128, 1], f32, tag="mh")
                nc.vector.reduce_max(out=mh[:], in_=MB[:], axis=X)
                T = small.tile([128, NU], f32, tag="T")
                nc.vector.tensor_scalar_sub(T[:], MB[:], mh[:])
                nc.vector.tensor_scalar_max(T[:], T[:], -80.0)
                E = small.tile([128, NU], f32, tag="E")
                nc.scalar.activation(out=E[:], in_=T[:], func=Act.Exp)
                scr4 = small.tile([128, NU], f32, tag="scr4")
                nc.vector.tensor_mul(scr4[:], S[:], E[:])
                nc.vector.reduce_sum(out=SS[:, t : t + 1], in_=scr4[:], axis=X)
                qT = small.tile([128, 1], f32, tag="qT")
                nc.vector.tensor_scalar_sub(qT[:], POS[:, t : t + 1], mh[:])
                nc.scalar.activation(
                    out=QQ[:, t : t + 1], in_=qT[:], func=Act.Exp
                )

            # ---- branch epilogue on [128, NT] ----
            D = small.tile([128, NT], f32, tag="D")
            nc.vector.tensor_add(D[:], SS[:], QQ[:])
            nc.vector.tensor_scalar_add(D[:], D[:], EPS)
            R = small.tile([128, NT], f32, tag="R")
            nc.vector.reciprocal(R[:], D[:])
            SIG = small.tile([128, NT], f32, tag="SIG")
            nc.vector.tensor_mul(SIG[:], QQ[:], R[:])
            LAM = small.tile([128, NT], f32, tag="LAM")
            nc.scalar.activation(
                out=LAM[:], in_=SIG[:], func=Act.Ln, bias=epsb[:], scale=1.0
            )
            A = small.tile([128, NT], f32, tag="A")
            nc.vector.tensor_scalar(
                out=A[:], in0=lgB[:], scalar1=POS_THRESH, scalar2=None,
                op0=Alu.is_gt,
            )
            W = small.tile([128, NT], f32, tag="W")
            nc.vector.tensor_tensor(W[:], lgA[:], lgB[:], op=Alu.is_lt)
            nc.vector.tensor_mul(W[:], W[:], A[:])
            scrN = small.tile([128, NT], f32, tag="scrN")
            nc.vector.tensor_mul(scrN[:], LAM[:], W[:])
            nc.vector.reduce_sum(
                out=outt[:, 2 * b : 2 * b + 1], in_=scrN[:], axis=X
            )
            nc.vector.reduce_sum(
                out=outt[:, 2 * b + 1 : 2 * b + 2], in_=W[:], axis=X
            )

        nc.sync.dma_start(out=out_d[:], in_=outt[:])

    nc.finalize()
    return nc


def _host_prep(inputs):
    bf = ml_dtypes.bfloat16
    f1 = np.ascontiguousarray(np.asarray(inputs["output_feat1"], np.float32))
    f2 = np.ascontiguousarray(np.asarray(inputs["output_feat2"], np.float32))
    l1 = np.asarray(inputs["pseudo_label1"], np.int32)
    l2 = np.asarray(inputs["pseudo_label2"], np.int32)
    g1 = np.asarray(inputs["pseudo_logits1"], np.float32)
    g2 = np.asarray(inputs["pseudo_logits2"], np.float32)
    ul1 = np.asarray(inputs["output_ul1"], np.float32)
    ul2 = np.asarray(inputs["output_ul2"], np.float32)
    i1 = np.asarray(inputs["selected_idx1"], np.int64)
    i2 = np.asarray(inputs["selected_idx2"], np.int64)

    b, c, h, w = ul1.shape
    u1 = ul1.transpose(0, 2, 3, 1).reshape(b * h * w, c)
    u2 = ul2.transpose(0, 2, 3, 1).reshape(b * h * w, c)
    mem = np.concatenate([u1[i1], u2[i2]], axis=0)               # [M, C]
    memlab = np.concatenate([l1[i1], l2[i2]], axis=0)            # [M]

    lab_eye = np.arange(NLAB, dtype=np.int32)
    oh_mem = (memlab[None, :] == lab_eye[:, None]).astype(np.float32)
    extmem = np.concatenate([mem.T / TEMP, oh_mem], 0).astype(bf)  # [277, M]

    def ext_anchor(f, lab):
        oh = (lab[None, :] == lab_eye[:, None]).astype(np.float32)
        return np.concatenate([f.T, -1000.0 * oh], 0).astype(bf)   # [277, N]

    ext1 = ext_anchor(f1, l1)
    ext2 = ext_anchor(f2, l2)

    def pack_rows(x):   # [RPC, C] -> [128, NT*C]
        return np.ascontiguousarray(
            x.reshape(NT, 128, C).transpose(1, 0, 2).reshape(128, NT * C)
        )

    def pack_vec(v):    # [RPC] -> [128, NT]
        return np.ascontiguousarray(v.reshape(NT, 128).T)

    in_maps = []
    for cix in range(NCORES):
        sl = slice(cix * RPC, (cix + 1) * RPC)
        in_maps.append({
            "ext1": np.ascontiguousarray(ext1[:, sl]),
            "ext2": np.ascontiguousarray(ext2[:, sl]),
            "extmem": extmem,
            "f1r": pack_rows((f1[sl] / TEMP).astype(bf)),
            "f2r": pack_rows(f2[sl].astype(bf)),
            "lg1": pack_vec(g1[sl]),
            "lg2": pack_vec(g2[sl]),
        })
    return in_maps


def _finalize(results):
    num1 = den1 = num2 = den2 = 0.0
    for r in results:
        o = np.asarray(r["out"], np.float64)
        num1 += o[:, 0].sum()
        den1 += o[:, 1].sum()
        num2 += o[:, 2].sum()
        den2 += o[:, 3].sum()
    loss = -(num1 / (den1 + 1e-12) + num2 / (den2 + 1e-12))
    return np.float32(loss)


def _run(inputs, trace=False):
    from concourse.bass_utils import run_bass_kernel_spmd

    if "nc" not in _cache:
        _cache["nc"] = _build()
    in_maps = _host_prep(inputs)
    res = run_bass_kernel_spmd(
        _cache["nc"], in_maps, list(range(NCORES)), trace=trace
    )
    return _finalize(res.results), res


def kernel(**inputs):
    out, _ = _run(inputs)
    return out


def kernel_with_profile(**inputs):
    out, res = _run(inputs, trace=True)
    return out, res
